# revision 8
# baseline (speedup 1.0000x reference)
"""ChannelTimeAttention Trainium2 kernel (v4).

Reference computation (per (b, c) pair, all independent):
    pooled = AdaptiveAvgPool(x[b, :, c]) -> [t, 8*8]      (7x7 block means)
    q = pooled @ Wq + bq ; k = pooled @ Wk + bk           [t, 32]
    att = softmax(q @ k.T / sqrt(t))                      [t, t]
    out[b, :, c] = att @ x[b, :, c].reshape(t, h*w)

Sharding: data-parallel over b — one batch element per NeuronCore (8 cores).
Each core streams its x slice [t=16, c=64, h=56, w=56] through SBUF once in
8 "packs" of 8 channels, partition layout (t*8 + c_local).  Per pack:
  DVE single-pass XY strided reduce     -> pooled sums [128, 8, 8]
  PE  transpose -> bf16 fused q|k matmul -> scores (full 128x128 cross)
  additive block-diag mask (-30), exp WITHOUT max-subtraction, 1/sum
  folded into the PSUM evacuations
  PE  transpose(e) -> block-diagonal lhsT; att@v in 7 N=448 f32r chunks
  DMA out.
1/49 (pool mean) and 1/sqrt(16) (score scale) are folded into Wq/bq/Wk on
host; q/k in bf16 is safe (~1e-4 rel err, dominated by f32r rounding).

DMA plan (v4) — measured HW model:
  * descriptors are per-partition (12.5 KiB);  a read desc costs ~790 ns,
    a write desc ~500 ns on each of the 16 shared DMA engines -> reads
    alone cap at ~220 GB/s; an independent concurrent write stream lifts
    the core to ~300 GB/s (measured 86-88 us for the 25.7 MiB round trip).
  * only ~4 DMA triggers per engine issue freely; the 5th stalls the
    ISSUING ENGINE on a queue-completion semaphore.  A stalled trigger on
    a compute engine blocks all compute queued behind it (this cost v3
    ~25 us), so: sync and scalar carry EXACTLY 4 input DMAs each and
    nothing else; every output rides the gpsimd SWDGE ring, where a
    stalled trigger only delays later output triggers.
  sync   (HWDGE): v0[0:64]  v2 v4 v6
  scalar (HWDGE): v0[64:128] v1 v3 v5   (v0 split -> pack 0 lands in half
    the time, so compute and the output write stream start early)
  gpsimd (SWDGE): consts, v7 (ahead of all outputs; drains ~15 us before
    o0's data is ready), then o0..o7.
Stage2 of pack p is emitted immediately after stage1 of pack p (NOT
offset): in-order engine queues then run the PSUM evacuations (and the
output DMA triggers) before the next pack's reduce, keeping the write
stream dense from ~18 us on.  The pooledT bias ones-rows are written once
up-front (explicit double buffer) so no per-pack gpsimd memset sits
between output triggers and stage1 work.
PE matmuls never read DMA-written weight tiles directly (waits on PE
instructions get merged onto cluster-head LDWEIGHTS with inflated DMA-lane
thresholds); weights are rematerialized through a DVE copy first.
"""

import numpy as np

B, T, C, H, W = 8, 16, 64, 56, 56
DS = 8
DIN = DS * DS  # 64
DOUT = 32
HW = H * W  # 3136
CG = 8  # channels per pack
NPACK = C // CG  # 8
P = CG * T  # 128 partitions
NCH = 7  # output free-dim chunks per pack
CHN = HW // NCH  # 448
N_CORES = 8
MASK_NEG = -30.0


def _build_nc():
    import concourse.bacc as bacc
    import concourse.tile as tile
    from concourse import mybir
    from contextlib import ExitStack

    f32 = mybir.dt.float32
    f32r = mybir.dt.float32r
    bf16 = mybir.dt.bfloat16
    nc = bacc.Bacc(trn_type="TRN2", num_swdge_queues=2)

    x_h = nc.dram_tensor("x", [T, C, H, W], f32, kind="ExternalInput")
    # all small constants packed into ONE [128, 452] array (one DMA):
    #   cols 128:160 wq_aug / 160:192 wk_aug (rows 0:65 — row 64 is the
    #   bias row, matched by a ones-row appended to pooledT so the bias add
    #   is folded into the q/k matmuls); cols 194:450 rows 32:40 are the
    #   scores-matmul augmentation rows ([indicator | -30*(1-indicator)]):
    #   8 extra contraction rows reproduce the block-diagonal -30 mask
    #   inside the scores matmul, so no separate DVE mask add is needed
    cn_h = nc.dram_tensor("consts", [P, 452], f32, kind="ExternalInput")
    out_h = nc.dram_tensor("out", [T, C, H, W], f32, kind="ExternalOutput")

    XY = mybir.AxisListType.XY
    Exp = mybir.ActivationFunctionType.Exp
    Copy = mybir.ActivationFunctionType.Copy

    with ExitStack() as ctx:
        tc = ctx.enter_context(tile.TileContext(nc))
        singles = ctx.enter_context(tc.tile_pool(name="singles", bufs=1))
        # bufs=NPACK: every v-DMA writes a fresh slot -> no WAW waits on DMAs
        vpool = ctx.enter_context(tc.tile_pool(name="vpool", bufs=NPACK))
        opool = ctx.enter_context(tc.tile_pool(name="opool", bufs=3))
        small = ctx.enter_context(tc.tile_pool(name="small", bufs=2))
        attpool = ctx.enter_context(tc.tile_pool(name="attpool", bufs=3))
        psA = ctx.enter_context(tc.tile_pool(name="psA", bufs=1, space="PSUM"))
        psB = ctx.enter_context(tc.tile_pool(name="psB", bufs=4, space="PSUM"))

        consts = singles.tile([P, 452], f32)
        # consts lead the gpsimd ring (tiny: ~2 KiB/partition)
        nc.gpsimd.dma_start(out=consts, in_=cn_h[:])
        ident = singles.tile([P, P], f32)
        identr = singles.tile([P, P], f32r)

        x_ap = x_h[:]
        out_ap = out_h[:]

        # Input DMAs all issued up-front.  t-MAJOR partition order
        # (partition = t*8 + c_l) so the DMA walks DRAM nearly sequentially.
        v_tiles = []
        for p in range(NPACK):
            c0 = p * CG
            v = vpool.tile([P, HW], f32r, tag="v")
            src = x_ap[:, c0 : c0 + CG, :, :].rearrange("t c h w -> t c (h w)")
            eng = nc.sync if p % 2 == 0 else nc.scalar
            eng.dma_start(out=v[:], in_=src.bitcast(f32r))
            v_tiles.append(v)

        # identity built on-chip (gpsimd memset + affine_select) — no DMA
        from concourse.masks import make_identity

        make_identity(nc, ident[:])
        nc.scalar.copy(identr, ident[:])

        # PE-consumed weights rematerialized through DVE (see module docstring)
        wqk = singles.tile([DIN + 1, DIN], bf16)
        nc.vector.tensor_copy(out=wqk, in_=consts[0 : DIN + 1, 128:192])
        # qk tiles are explicit (not pooled) so the mask-augmentation rows
        # 32:40 can be written ONCE; rows 0:32 rotate per pack (p%2)
        QKR = DOUT + CG  # 40 contraction rows for the scores matmul
        qk_ab = [
            singles.tile([QKR, 2 * P], bf16, name=f"qk{i}", tag=f"qk{i}")
            for i in range(2)
        ]
        for t in qk_ab:
            nc.vector.tensor_copy(out=t[DOUT:QKR, :], in_=consts[DOUT:QKR, 194:450])
        # pooledT double buffer, explicit so the bias ones-row (row 64,
        # multiplying the weight-matrix bias row) is written ONCE here and
        # never touched again — keeps per-pack gpsimd work off the loop
        pooledT_ab = [
            singles.tile([DIN + 1, P], bf16, name=f"pooledT{i}", tag=f"pooledT{i}")
            for i in range(2)
        ]
        for t in pooledT_ab:
            nc.gpsimd.memset(t[DIN : DIN + 1, :], 1.0)

        def emit_stage1(p):
            v = v_tiles[p]
            # ---- adaptive avg pool, single strided XY reduce ----
            # hw = (i*7+u)*56 + (j*7+vv); reduce (u, vv) -> pooled[p, i, j]
            # tile_wait_until feeds the STATIC scheduler the measured v-tile
            # arrival time (~7 us trigger + ~6.5 us/tile stream): its DMA
            # model is optimistic, and without this it packs all 8 reduces
            # back-to-back on DVE, pushing the PSUM evacuations (and thus
            # every output DMA) ~30 us late.
            pooled = small.tile([P, DS, DS], f32, tag="pooled")
            with tc.tile_wait_until(0.019 + 0.007 * p):
                nc.vector.reduce_sum(
                    out=pooled[:],
                    in_=v[:].bitcast(f32).rearrange(
                        "p (i u j vv) -> p i j u vv", i=DS, u=7, j=DS, vv=7
                    ),
                    axis=XY,
                )

            # ---- pooled^T via PE so the q|k matmul contracts over d_in ----
            pooledT_ps = psA.tile([DIN, P], f32, tag="pooledT_ps")
            nc.tensor.transpose(
                pooledT_ps,
                pooled[:].rearrange("p i j -> p (i j)"),
                ident[:],
            )
            pooledT = pooledT_ab[p % 2]
            nc.scalar.copy(pooledT[0:DIN, :], pooledT_ps)

            # ---- q^T, k^T [32, 128] into ONE PSUM bank (bf16: 1 inst +
            # 1 cyc/col); bias comes along via the augmented ones-row ----
            qkT_ps = psA.tile([DOUT, 2 * P], f32, tag="qkT_ps")
            nc.tensor.matmul(
                qkT_ps[:, 0:P], lhsT=wqk[:, 0:DOUT], rhs=pooledT[:],
                start=True, stop=True,
            )
            nc.tensor.matmul(
                qkT_ps[:, P : 2 * P], lhsT=wqk[:, DOUT : 2 * DOUT],
                rhs=pooledT[:], start=True, stop=True,
            )
            qk = qk_ab[p % 2]
            nc.scalar.copy(qk[0:DOUT, :], qkT_ps)

            # ---- full cross scores [128, 128] with the -30 block-diagonal
            # mask folded in via the 8 augmentation contraction rows ----
            sc_ps = psA.tile([P, P], f32, tag="sc_ps")
            nc.tensor.matmul(
                sc_ps, lhsT=qk[:, 0:P], rhs=qk[:, P : 2 * P],
                start=True, stop=True,
            )

            # ---- exp straight from PSUM (scores ~1e-6 + mask -30: no
            # max-subtraction needed); 1/sum is folded into the evacuations
            e = small.tile([P, P], f32r, tag="e")
            ssum = small.tile([P, 1], f32, tag="ssum")
            nc.scalar.activation(out=e, in_=sc_ps, func=Exp, accum_out=ssum)
            rinv = attpool.tile([P, 1], f32, tag="rinv")
            nc.vector.reciprocal(rinv, ssum)

            # ---- e^T (block-diagonal) becomes the stationary operand ----
            attT_ps = psA.tile([P, P], f32r, tag="attT_ps")
            nc.tensor.transpose(attT_ps, e[:], identr[:])
            attT = attpool.tile([P, P], f32r, tag="attT")
            nc.scalar.copy(attT, attT_ps)
            return attT, rinv

        def emit_stage2(p, attT, rinv):
            c0 = p * CG
            v = v_tiles[p]
            # high_priority: stage2 must win scheduler ties against later
            # packs' stage1 chains — otherwise PE runs 2-3 transpose/scores
            # chains ahead and every evacuation (and output DMA) slips
            with tc.high_priority():
                o = opool.tile([P, HW], f32, tag="o")
                # claim the o slot with a cheap op: absorbs the WAR wait on
                # the out-DMA that previously read this slot
                nc.gpsimd.memset(o[:, 0:1], 0.0)
                for ch in range(NCH):
                    sl = slice(ch * CHN, (ch + 1) * CHN)
                    ops = psB.tile([P, CHN], f32, tag="ochunk")
                    nc.tensor.matmul(
                        ops,
                        lhsT=attT[:],
                        rhs=v[:, sl],
                        start=True,
                        stop=True,
                    )
                    # evacuation multiplies by 1/sum (softmax normalization).
                    # ALL on ACT (~6 us/pack, fits the ~10 us duplex pack
                    # period): GpSimd cannot read PSUM, and on DVE the
                    # scheduler displaces evacs behind later packs' reduces
                    nc.scalar.activation(
                        out=o[:, sl], in_=ops, func=Copy, scale=rinv
                    )

                dst = out_ap[:, c0 : c0 + CG, :, :].rearrange(
                    "t c h w -> t c (h w)"
                )
                # ALL outputs ride the gpsimd SWDGE ring: the input rings
                # stay read-only (no FIFO head-of-line blocking) while the
                # 16 DMA engines interleave read+write descs (duplex ~300)
                nc.gpsimd.dma_start(out=dst, in_=o[:])

        for p in range(NPACK):
            attT, rinv = emit_stage1(p)
            emit_stage2(p, attT, rinv)

    nc.compile()
    return nc


def _host_consts(Wq, bq, Wk, bk):
    # fold pool-mean 1/49 into both weight mats; fold score 1/sqrt(t)=1/4
    # into the q side (weights AND bias)
    wq_eff = (Wq / (49.0 * 4.0)).astype(np.float32)
    bq_eff = (bq / 4.0).astype(np.float32)
    wk_eff = (Wk / 49.0).astype(np.float32)
    bk_eff = bk.astype(np.float32)
    # t-major partition order: row i = (t=i//8, c=i%8); attention pairs
    # (i, j) belong to the same channel iff i%8 == j%8.  The mask reaches
    # the scores through 8 augmentation rows: q side carries the channel
    # indicator, k side carries the per-channel mask columns.
    idx = np.arange(P)
    ind = (np.arange(CG)[:, None] == (idx % CG)[None, :]).astype(np.float32)
    consts = np.zeros((P, 452), dtype=np.float32)
    consts[0:DIN, 128:160] = wq_eff
    consts[0:DIN, 160:192] = wk_eff
    consts[DIN, 128:160] = bq_eff
    consts[DIN, 160:192] = bk_eff
    consts[DOUT : DOUT + CG, 194:322] = ind
    consts[DOUT : DOUT + CG, 322:450] = MASK_NEG * (1.0 - ind)
    return consts


def kernel(x, Wq, bq, Wk, bk):
    from concourse.bass_utils import run_bass_kernel_spmd

    x = np.ascontiguousarray(x, dtype=np.float32)
    consts = _host_consts(Wq, bq, Wk, bk)

    nc = _build_nc()
    in_maps = [{"x": x[i], "consts": consts} for i in range(N_CORES)]
    res = run_bass_kernel_spmd(nc, in_maps, core_ids=list(range(N_CORES)))
    global LAST_RUN
    LAST_RUN = res
    out = np.stack([r["out"] for r in res.results], axis=0)
    return out


LAST_RUN = None


# revision 12
# speedup vs baseline: 1.2513x; 1.2513x over previous
"""ChannelTimeAttention Trainium2 kernel (v7).

Reference computation (per (b, c) pair, all independent):
    pooled = AdaptiveAvgPool(x[b, :, c]) -> [t, 8*8]      (7x7 block means)
    q = pooled @ Wq + bq ; k = pooled @ Wk + bk           [t, 32]
    att = softmax(q @ k.T / sqrt(t))                      [t, t]
    out[b, :, c] = att @ x[b, :, c].reshape(t, h*w)

Sharding: data-parallel over b — one batch element per NeuronCore (8 cores).
Each core streams its x slice [t=16, c=64, h=56, w=56] through SBUF once in
8 "packs" of 8 channels, partition layout (t*8 + c_local).  Per pack:
  DVE single-pass XY strided reduce     -> pooled sums [128, 8, 8]
  PE  transpose -> bf16 fused q|k matmul -> scores TRANSPOSED (lhsT/rhs
  swapped, so exp() directly yields e^T, the stationary operand of att@v —
  no separate PE transpose / copy of the attention matrix is needed)
  additive block-diag mask (-30) folded into the scoresT matmul via 8
  augmentation contraction rows; exp WITHOUT max-subtraction (scores are
  ~1e-6 so exp never overflows); softmax denominators = column sums of e^T
  from a 1-column PE matmul against ones; 1/sum folded into the PSUM
  evacuations; att@v in 7 N=448 f32r chunks, v fed straight from the
  DMA'd f32 tile via bitcast; DMA out.
1/49 (pool mean) and 1/sqrt(16) (score scale) are folded into Wq/bq/Wk on
host; q/k in bf16 is safe (~1e-4 rel err, dominated by f32r rounding).

DMA plan — measured HW model: descriptors are per-partition (12.5 KiB);
a read desc costs ~790 ns and a write desc ~500 ns on each of the 16
shared DMA engines, so reads alone cap at ~220 GB/s and a concurrent
independent write stream lifts the core to ~300 GB/s.  Only ~4 DMA
triggers per engine issue freely; the 5th stalls the ISSUING ENGINE, so
sync and scalar carry EXACTLY the 4 input DMAs each and nothing else;
every output rides the gpsimd SWDGE ring.

Engine assignment is chosen so the Tile static scheduler cannot starve
the output stream: DVE owns stage1 (reduce + the pooledT/qk PSUM-copies);
ACT owns exp + ALL PSUM evacuations (its only stage1 op is the 0.4 us
exp, so evacuations are never displaced by later packs' stage1 work);
stage2 is emitted under high_priority and reduces carry tile_wait_until
arrival hints so the simulated schedule matches measured DMA pacing.
PE matmuls never read DMA-written weight tiles directly (waits on PE
instructions get merged onto cluster-head LDWEIGHTS with inflated DMA-lane
thresholds); weights are rematerialized through a DVE copy first.
"""

import numpy as np

B, T, C, H, W = 8, 16, 64, 56, 56
DS = 8
DIN = DS * DS  # 64
DOUT = 32
HW = H * W  # 3136
CG = 8  # channels per pack
NPACK = C // CG  # 8
P = CG * T  # 128 partitions
NCH = 7  # output free-dim chunks per pack
CHN = HW // NCH  # 448
N_CORES = 8
MASK_NEG = -30.0


def _build_nc():
    import concourse.bacc as bacc
    import concourse.tile as tile
    from concourse import mybir
    from contextlib import ExitStack

    f32 = mybir.dt.float32
    f32r = mybir.dt.float32r
    bf16 = mybir.dt.bfloat16
    nc = bacc.Bacc(trn_type="TRN2", num_swdge_queues=2)

    x_h = nc.dram_tensor("x", [T, C, H, W], f32, kind="ExternalInput")
    # all small constants packed into ONE [128, 452] array (one DMA):
    #   cols 128:160 wq_aug / 160:192 wk_aug (rows 0:65 — row 64 is the
    #   bias row, matched by a ones-row appended to pooledT so the bias add
    #   is folded into the q/k matmuls); cols 194:450 rows 32:40 are the
    #   scoresT-matmul augmentation rows: the K side (lhsT) carries the
    #   channel indicator, the Q side (rhs) carries -30*(1-indicator), so
    #   the 8 extra contraction rows reproduce the block-diagonal -30 mask
    #   inside the scoresT matmul; col 450 is a ones-column (denominator
    #   matmul rhs); col 451 unused
    cn_h = nc.dram_tensor("consts", [P, 452], f32, kind="ExternalInput")
    out_h = nc.dram_tensor("out", [T, C, H, W], f32, kind="ExternalOutput")

    XY = mybir.AxisListType.XY
    Exp = mybir.ActivationFunctionType.Exp
    Copy = mybir.ActivationFunctionType.Copy

    with ExitStack() as ctx:
        tc = ctx.enter_context(tile.TileContext(nc))
        singles = ctx.enter_context(tc.tile_pool(name="singles", bufs=1))
        # bufs=NPACK: every v-DMA writes a fresh slot -> no WAW waits on DMAs
        vpool = ctx.enter_context(tc.tile_pool(name="vpool", bufs=NPACK))
        opool = ctx.enter_context(tc.tile_pool(name="opool", bufs=3))
        small = ctx.enter_context(tc.tile_pool(name="small", bufs=2))
        epool = ctx.enter_context(tc.tile_pool(name="epool", bufs=2))
        psA = ctx.enter_context(tc.tile_pool(name="psA", bufs=1, space="PSUM"))
        psB = ctx.enter_context(tc.tile_pool(name="psB", bufs=4, space="PSUM"))
        psS = ctx.enter_context(tc.tile_pool(name="psS", bufs=1, space="PSUM"))

        consts = singles.tile([P, 452], f32)
        # consts lead the gpsimd ring (tiny: ~2 KiB/partition)
        nc.gpsimd.dma_start(out=consts, in_=cn_h[:])
        ident = singles.tile([P, P], f32)

        x_ap = x_h[:]
        out_ap = out_h[:]

        # Input DMAs all issued up-front.  t-MAJOR partition order
        # (partition = t*8 + c_l) so the DMA walks DRAM nearly sequentially.
        v_tiles = []
        for p in range(NPACK):
            c0 = p * CG
            v = vpool.tile([P, HW], f32r, tag="v")
            src = x_ap[:, c0 : c0 + CG, :, :].rearrange("t c h w -> t c (h w)")
            eng = nc.sync if p % 2 == 0 else nc.scalar
            eng.dma_start(out=v[:], in_=src.bitcast(f32r))
            v_tiles.append(v)

        # identity built on-chip (gpsimd memset + affine_select) — no DMA
        from concourse.masks import make_identity

        make_identity(nc, ident[:])

        # PE-consumed weights rematerialized through DVE (see module docstring)
        wqk = singles.tile([DIN + 1, DIN], bf16)
        nc.vector.tensor_copy(out=wqk, in_=consts[0 : DIN + 1, 128:192])
        # f32r matmuls need a wide moving operand (s3d3_mm_fp32r
        # restrictions reject N=1), so the denominator matmul uses a
        # 256-wide ones tile and the reciprocal reads column 0
        onescol = singles.tile([P, 256], f32)
        nc.gpsimd.memset(onescol[:, 0:256], 1.0)
        # qk tiles are explicit (not pooled) so the mask-augmentation rows
        # 32:40 can be written ONCE; rows 0:32 rotate per pack (p%2)
        QKR = DOUT + CG  # 40 contraction rows for the scoresT matmul
        qk_ab = [
            singles.tile([QKR, 2 * P], bf16, name=f"qk{i}", tag=f"qk{i}")
            for i in range(2)
        ]
        for t in qk_ab:
            nc.vector.tensor_copy(out=t[DOUT:QKR, :], in_=consts[DOUT:QKR, 194:450])
        # pooledT double buffer, explicit so the bias ones-row (row 64,
        # multiplying the weight-matrix bias row) is written ONCE here
        pooledT_ab = [
            singles.tile([DIN + 1, P], bf16, name=f"pooledT{i}", tag=f"pooledT{i}")
            for i in range(2)
        ]
        for t in pooledT_ab:
            nc.gpsimd.memset(t[DIN : DIN + 1, :], 1.0)

        def emit_stage1(p):
            v = v_tiles[p]
            # ---- adaptive avg pool, single strided XY reduce ----
            # hw = (i*7+u)*56 + (j*7+vv); reduce (u, vv) -> pooled[p, i, j]
            # tile_wait_until feeds the STATIC scheduler the measured v-tile
            # arrival time (~7 us trigger + ~7 us/tile duplex stream): its
            # DMA model is optimistic, and without this it packs all 8
            # reduces back-to-back on DVE, pushing everything late.
            pooled = small.tile([P, DS, DS], f32, tag="pooled")
            with tc.tile_wait_until(0.019 + 0.007 * p):
                nc.vector.reduce_sum(
                    out=pooled[:],
                    in_=v[:].bitcast(f32).rearrange(
                        "p (i u j vv) -> p i j u vv", i=DS, u=7, j=DS, vv=7
                    ),
                    axis=XY,
                )

            # ---- pooled^T via PE so the q|k matmul contracts over d_in ----
            pooledT_ps = psA.tile([DIN, P], f32, tag="pooledT_ps")
            nc.tensor.transpose(
                pooledT_ps,
                pooled[:].rearrange("p i j -> p (i j)"),
                ident[:],
            )
            pooledT = pooledT_ab[p % 2]
            nc.vector.tensor_copy(out=pooledT[0:DIN, :], in_=pooledT_ps[:])

            # ---- q^T, k^T [32, 128] into ONE PSUM bank (bf16: 1 inst +
            # 1 cyc/col); bias comes along via the augmented ones-row ----
            qkT_ps = psA.tile([DOUT, 2 * P], f32, tag="qkT_ps")
            nc.tensor.matmul(
                qkT_ps[:, 0:P], lhsT=wqk[:, 0:DOUT], rhs=pooledT[:],
                start=True, stop=True,
            )
            nc.tensor.matmul(
                qkT_ps[:, P : 2 * P], lhsT=wqk[:, DOUT : 2 * DOUT],
                rhs=pooledT[:], start=True, stop=True,
            )
            qk = qk_ab[p % 2]
            nc.vector.tensor_copy(out=qk[0:DOUT, :], in_=qkT_ps[:])

            # ---- scores TRANSPOSED [s, t2]: lhsT = K side (with indicator
            # aug rows), rhs = Q side (with mask aug rows).  exp of this is
            # e^T, directly the stationary operand of att@v ----
            sc_ps = psA.tile([P, P], f32, tag="sc_ps")
            nc.tensor.matmul(
                sc_ps, lhsT=qk[:, P : 2 * P], rhs=qk[:, 0:P],
                start=True, stop=True,
            )

            # ---- exp straight from PSUM (scores ~1e-6 + mask -30: no
            # max-subtraction needed) ----
            eT = epool.tile([P, P], f32r, tag="eT")
            nc.scalar.activation(out=eT, in_=sc_ps, func=Exp)
            return eT

        def emit_stage2(p, eT):
            c0 = p * CG
            v = v_tiles[p]
            # high_priority: stage2 must win scheduler ties against later
            # packs' stage1 chains — otherwise PE runs 2-3 transpose/scores
            # chains ahead and every evacuation (and output DMA) slips
            with tc.high_priority():
                # ---- softmax denominators: column sums of e^T via a
                # 1-column matmul against ones; reciprocal on DVE ----
                ssum_ps = psS.tile([P, 256], f32, tag="ssum_ps")
                nc.tensor.matmul(
                    ssum_ps, lhsT=eT[:], rhs=onescol[:].bitcast(f32r),
                    start=True, stop=True
                )
                rinv = small.tile([P, 1], f32, tag="rinv")
                nc.vector.reciprocal(rinv, ssum_ps[:, 0:1])

                o = opool.tile([P, HW], f32, tag="o")
                # claim the o slot with a cheap op: absorbs the WAR wait on
                # the out-DMA that previously read this slot
                nc.gpsimd.memset(o[:, 0:1], 0.0)
                for ch in range(NCH):
                    sl = slice(ch * CHN, (ch + 1) * CHN)
                    ops = psB.tile([P, CHN], f32, tag="ochunk")
                    nc.tensor.matmul(
                        ops,
                        lhsT=eT[:],
                        rhs=v[:, sl],
                        start=True,
                        stop=True,
                    )
                    # evacuation multiplies by 1/sum (softmax normalization);
                    # all on ACT, whose only stage1 duty is the cheap exp
                    nc.scalar.activation(
                        out=o[:, sl], in_=ops, func=Copy, scale=rinv
                    )

                dst = out_ap[:, c0 : c0 + CG, :, :].rearrange(
                    "t c h w -> t c (h w)"
                )
                # ALL outputs ride the gpsimd SWDGE ring: the input rings
                # stay read-only (no FIFO head-of-line blocking) while the
                # 16 DMA engines interleave read+write descs (duplex ~300)
                nc.gpsimd.dma_start(out=dst, in_=o[:])

        for p in range(NPACK):
            eT = emit_stage1(p)
            emit_stage2(p, eT)

    nc.compile()
    return nc


def _host_consts(Wq, bq, Wk, bk):
    # fold pool-mean 1/49 into both weight mats; fold score 1/sqrt(t)=1/4
    # into the q side (weights AND bias)
    wq_eff = (Wq / (49.0 * 4.0)).astype(np.float32)
    bq_eff = (bq / 4.0).astype(np.float32)
    wk_eff = (Wk / 49.0).astype(np.float32)
    bk_eff = bk.astype(np.float32)
    # t-major partition order: row i = (t=i//8, c=i%8); attention pairs
    # (i, j) belong to the same channel iff i%8 == j%8.  The mask reaches
    # scoresT through 8 augmentation rows: the K side (lhsT) carries the
    # channel indicator, the Q side (rhs) carries the per-channel -30 mask.
    idx = np.arange(P)
    ind = (np.arange(CG)[:, None] == (idx % CG)[None, :]).astype(np.float32)
    consts = np.zeros((P, 452), dtype=np.float32)
    consts[0:DIN, 128:160] = wq_eff
    consts[0:DIN, 160:192] = wk_eff
    consts[DIN, 128:160] = bq_eff
    consts[DIN, 160:192] = bk_eff
    consts[DOUT : DOUT + CG, 194:322] = MASK_NEG * (1.0 - ind)  # Q-side aug
    consts[DOUT : DOUT + CG, 322:450] = ind                      # K-side aug
    consts[:, 450] = 1.0                                         # ones column
    return consts


def kernel(x, Wq, bq, Wk, bk):
    from concourse.bass_utils import run_bass_kernel_spmd

    x = np.ascontiguousarray(x, dtype=np.float32)
    consts = _host_consts(Wq, bq, Wk, bk)

    nc = _build_nc()
    in_maps = [{"x": x[i], "consts": consts} for i in range(N_CORES)]
    res = run_bass_kernel_spmd(nc, in_maps, core_ids=list(range(N_CORES)))
    global LAST_RUN
    LAST_RUN = res
    out = np.stack([r["out"] for r in res.results], axis=0)
    return out


LAST_RUN = None


# revision 13
# speedup vs baseline: 1.3130x; 1.0493x over previous
"""ChannelTimeAttention Trainium2 kernel (v7).

Reference computation (per (b, c) pair, all independent):
    pooled = AdaptiveAvgPool(x[b, :, c]) -> [t, 8*8]      (7x7 block means)
    q = pooled @ Wq + bq ; k = pooled @ Wk + bk           [t, 32]
    att = softmax(q @ k.T / sqrt(t))                      [t, t]
    out[b, :, c] = att @ x[b, :, c].reshape(t, h*w)

Sharding: data-parallel over b — one batch element per NeuronCore (8 cores).
Each core streams its x slice [t=16, c=64, h=56, w=56] through SBUF once in
8 "packs" of 8 channels, partition layout (t*8 + c_local).  Per pack:
  DVE single-pass XY strided reduce     -> pooled sums [128, 8, 8]
  PE  transpose -> bf16 fused q|k matmul -> scores TRANSPOSED (lhsT/rhs
  swapped, so exp() directly yields e^T, the stationary operand of att@v —
  no separate PE transpose / copy of the attention matrix is needed)
  additive block-diag mask (-30) folded into the scoresT matmul via 8
  augmentation contraction rows; exp WITHOUT max-subtraction (scores are
  ~1e-6 so exp never overflows); softmax denominators = column sums of e^T
  from a 1-column PE matmul against ones; 1/sum folded into the PSUM
  evacuations; att@v in 7 N=448 f32r chunks, v fed straight from the
  DMA'd f32 tile via bitcast; DMA out.
1/49 (pool mean) and 1/sqrt(16) (score scale) are folded into Wq/bq/Wk on
host; q/k in bf16 is safe (~1e-4 rel err, dominated by f32r rounding).

DMA plan — measured HW model: descriptors are per-partition (12.5 KiB);
a read desc costs ~790 ns and a write desc ~500 ns on each of the 16
shared DMA engines, so reads alone cap at ~220 GB/s and a concurrent
independent write stream lifts the core to ~300 GB/s.  Only ~4 DMA
triggers per engine issue freely; the 5th stalls the ISSUING ENGINE, so
sync and scalar carry EXACTLY the 4 input DMAs each and nothing else;
every output rides the gpsimd SWDGE ring.

Engine assignment is chosen so the Tile static scheduler cannot starve
the output stream: DVE owns stage1 (reduce + the pooledT/qk PSUM-copies);
ACT owns exp + ALL PSUM evacuations (its only stage1 op is the 0.4 us
exp, so evacuations are never displaced by later packs' stage1 work);
stage2 is emitted under high_priority and reduces carry tile_wait_until
arrival hints so the simulated schedule matches measured DMA pacing.
PE matmuls never read DMA-written weight tiles directly (waits on PE
instructions get merged onto cluster-head LDWEIGHTS with inflated DMA-lane
thresholds); weights are rematerialized through a DVE copy first.
"""

import numpy as np

B, T, C, H, W = 8, 16, 64, 56, 56
DS = 8
DIN = DS * DS  # 64
DOUT = 32
HW = H * W  # 3136
CG = 8  # channels per pack
NPACK = C // CG  # 8
P = CG * T  # 128 partitions
NCH = 7  # output free-dim chunks per pack
CHN = HW // NCH  # 448
N_CORES = 8
MASK_NEG = -30.0


def _build_nc():
    import concourse.bacc as bacc
    import concourse.tile as tile
    from concourse import mybir
    from contextlib import ExitStack

    f32 = mybir.dt.float32
    f32r = mybir.dt.float32r
    bf16 = mybir.dt.bfloat16
    nc = bacc.Bacc(trn_type="TRN2", num_swdge_queues=2)

    x_h = nc.dram_tensor("x", [T, C, H, W], f32, kind="ExternalInput")
    # all small constants packed into ONE [128, 452] array (one DMA):
    #   cols 128:160 wq_aug / 160:192 wk_aug (rows 0:65 — row 64 is the
    #   bias row, matched by a ones-row appended to pooledT so the bias add
    #   is folded into the q/k matmuls); cols 194:450 rows 32:40 are the
    #   scoresT-matmul augmentation rows: the K side (lhsT) carries the
    #   channel indicator, the Q side (rhs) carries -30*(1-indicator), so
    #   the 8 extra contraction rows reproduce the block-diagonal -30 mask
    #   inside the scoresT matmul; col 450 is a ones-column (denominator
    #   matmul rhs); col 451 unused
    cn_h = nc.dram_tensor("consts", [P, 452], f32, kind="ExternalInput")
    out_h = nc.dram_tensor("out", [T, C, H, W], f32, kind="ExternalOutput")

    XY = mybir.AxisListType.XY
    Exp = mybir.ActivationFunctionType.Exp
    Copy = mybir.ActivationFunctionType.Copy

    with ExitStack() as ctx:
        tc = ctx.enter_context(tile.TileContext(nc))
        singles = ctx.enter_context(tc.tile_pool(name="singles", bufs=1))
        # bufs=NPACK: every v-DMA writes a fresh slot -> no WAW waits on DMAs
        vpool = ctx.enter_context(tc.tile_pool(name="vpool", bufs=NPACK))
        opool = ctx.enter_context(tc.tile_pool(name="opool", bufs=3))
        small = ctx.enter_context(tc.tile_pool(name="small", bufs=2))
        epool = ctx.enter_context(tc.tile_pool(name="epool", bufs=2))
        # ONE shared bank for pooledT_ps/qkT_ps/ssum_ps (their lifetimes
        # are sequential): the bank's WAR rotation forces pack p+1's PE
        # transpose to wait for pack p's denominator reciprocal, which
        # structurally stops the scheduler from running stage1 chains 2-3
        # packs ahead of att@v (that front-running idled ACT ~13 us).
        # sc_ps gets its own bank; the 6 remaining banks buffer att@v
        # chunks so no chunk matmul waits on the slowest evacuation.
        psA = ctx.enter_context(tc.tile_pool(name="psA", bufs=1, space="PSUM"))
        psB = ctx.enter_context(tc.tile_pool(name="psB", bufs=6, space="PSUM"))

        consts = singles.tile([P, 452], f32)
        # consts lead the gpsimd ring (tiny: ~2 KiB/partition)
        nc.gpsimd.dma_start(out=consts, in_=cn_h[:])
        ident = singles.tile([P, P], f32)

        x_ap = x_h[:]
        out_ap = out_h[:]

        # Input DMAs all issued up-front.  t-MAJOR partition order
        # (partition = t*8 + c_l) so the DMA walks DRAM nearly sequentially.
        v_tiles = []
        for p in range(NPACK):
            c0 = p * CG
            v = vpool.tile([P, HW], f32r, tag="v")
            src = x_ap[:, c0 : c0 + CG, :, :].rearrange("t c h w -> t c (h w)")
            eng = nc.sync if p % 2 == 0 else nc.scalar
            eng.dma_start(out=v[:], in_=src.bitcast(f32r))
            v_tiles.append(v)

        # identity built on-chip (gpsimd memset + affine_select) — no DMA
        from concourse.masks import make_identity

        make_identity(nc, ident[:])

        # PE-consumed weights rematerialized through DVE (see module docstring)
        wqk = singles.tile([DIN + 1, DIN], bf16)
        nc.vector.tensor_copy(out=wqk, in_=consts[0 : DIN + 1, 128:192])
        # f32r matmuls need a wide moving operand (s3d3_mm_fp32r
        # restrictions reject N=1), so the denominator matmul uses a
        # 256-wide ones tile and the reciprocal reads column 0
        onescol = singles.tile([P, 256], f32)
        nc.gpsimd.memset(onescol[:, 0:256], 1.0)
        # qk tiles are explicit (not pooled) so the mask-augmentation rows
        # 32:40 can be written ONCE; rows 0:32 rotate per pack (p%2)
        QKR = DOUT + CG  # 40 contraction rows for the scoresT matmul
        qk_ab = [
            singles.tile([QKR, 2 * P], bf16, name=f"qk{i}", tag=f"qk{i}")
            for i in range(2)
        ]
        for t in qk_ab:
            nc.vector.tensor_copy(out=t[DOUT:QKR, :], in_=consts[DOUT:QKR, 194:450])
        # pooledT double buffer, explicit so the bias ones-row (row 64,
        # multiplying the weight-matrix bias row) is written ONCE here
        pooledT_ab = [
            singles.tile([DIN + 1, P], bf16, name=f"pooledT{i}", tag=f"pooledT{i}")
            for i in range(2)
        ]
        for t in pooledT_ab:
            nc.gpsimd.memset(t[DIN : DIN + 1, :], 1.0)

        def emit_stage1(p):
            v = v_tiles[p]
            # ---- adaptive avg pool, single strided XY reduce ----
            # hw = (i*7+u)*56 + (j*7+vv); reduce (u, vv) -> pooled[p, i, j]
            # tile_wait_until feeds the STATIC scheduler the measured v-tile
            # arrival time (~7 us trigger + ~7 us/tile duplex stream): its
            # DMA model is optimistic, and without this it packs all 8
            # reduces back-to-back on DVE, pushing everything late.
            pooled = small.tile([P, DS, DS], f32, tag="pooled")
            with tc.tile_wait_until(0.019 + 0.007 * p):
                nc.vector.reduce_sum(
                    out=pooled[:],
                    in_=v[:].bitcast(f32).rearrange(
                        "p (i u j vv) -> p i j u vv", i=DS, u=7, j=DS, vv=7
                    ),
                    axis=XY,
                )

            # ---- pooled^T via PE so the q|k matmul contracts over d_in ----
            pooledT_ps = psA.tile([DIN, P], f32, tag="mix")
            nc.tensor.transpose(
                pooledT_ps,
                pooled[:].rearrange("p i j -> p (i j)"),
                ident[:],
            )
            pooledT = pooledT_ab[p % 2]
            nc.vector.tensor_copy(out=pooledT[0:DIN, :], in_=pooledT_ps[:])

            # ---- q^T, k^T [32, 128] into ONE PSUM bank (bf16: 1 inst +
            # 1 cyc/col); bias comes along via the augmented ones-row ----
            qkT_ps = psA.tile([DOUT, 2 * P], f32, tag="mix")
            nc.tensor.matmul(
                qkT_ps[:, 0:P], lhsT=wqk[:, 0:DOUT], rhs=pooledT[:],
                start=True, stop=True,
            )
            nc.tensor.matmul(
                qkT_ps[:, P : 2 * P], lhsT=wqk[:, DOUT : 2 * DOUT],
                rhs=pooledT[:], start=True, stop=True,
            )
            qk = qk_ab[p % 2]
            nc.vector.tensor_copy(out=qk[0:DOUT, :], in_=qkT_ps[:])

            # ---- scores TRANSPOSED [s, t2]: lhsT = K side (with indicator
            # aug rows), rhs = Q side (with mask aug rows).  exp of this is
            # e^T, directly the stationary operand of att@v ----
            sc_ps = psA.tile([P, P], f32, tag="sc_ps")
            nc.tensor.matmul(
                sc_ps, lhsT=qk[:, P : 2 * P], rhs=qk[:, 0:P],
                start=True, stop=True,
            )

            # ---- exp straight from PSUM (scores ~1e-6 + mask -30: no
            # max-subtraction needed) ----
            eT = epool.tile([P, P], f32r, tag="eT")
            nc.scalar.activation(out=eT, in_=sc_ps, func=Exp)
            return eT

        def emit_stage2(p, eT):
            c0 = p * CG
            v = v_tiles[p]
            # high_priority: stage2 must win scheduler ties against later
            # packs' stage1 chains — otherwise PE runs 2-3 transpose/scores
            # chains ahead and every evacuation (and output DMA) slips
            with tc.high_priority():
                # ---- softmax denominators: column sums of e^T via a
                # 1-column matmul against ones; reciprocal on DVE ----
                ssum_ps = psA.tile([P, 256], f32, tag="mix")
                nc.tensor.matmul(
                    ssum_ps, lhsT=eT[:], rhs=onescol[:].bitcast(f32r),
                    start=True, stop=True
                )
                rinv = small.tile([P, 1], f32, tag="rinv")
                nc.vector.reciprocal(rinv, ssum_ps[:, 0:1])

                o = opool.tile([P, HW], f32, tag="o")
                # claim the o slot with a cheap op: absorbs the WAR wait on
                # the out-DMA that previously read this slot
                nc.gpsimd.memset(o[:, 0:1], 0.0)
                for ch in range(NCH):
                    sl = slice(ch * CHN, (ch + 1) * CHN)
                    ops = psB.tile([P, CHN], f32, tag="ochunk")
                    nc.tensor.matmul(
                        ops,
                        lhsT=eT[:],
                        rhs=v[:, sl],
                        start=True,
                        stop=True,
                    )
                    # evacuation multiplies by 1/sum (softmax normalization);
                    # all on ACT, whose only stage1 duty is the cheap exp
                    nc.scalar.activation(
                        out=o[:, sl], in_=ops, func=Copy, scale=rinv
                    )

                dst = out_ap[:, c0 : c0 + CG, :, :].rearrange(
                    "t c h w -> t c (h w)"
                )
                # ALL outputs ride the gpsimd SWDGE ring: the input rings
                # stay read-only (no FIFO head-of-line blocking) while the
                # 16 DMA engines interleave read+write descs (duplex ~300)
                nc.gpsimd.dma_start(out=dst, in_=o[:])

        for p in range(NPACK):
            eT = emit_stage1(p)
            emit_stage2(p, eT)

    nc.compile()
    return nc


def _host_consts(Wq, bq, Wk, bk):
    # fold pool-mean 1/49 into both weight mats; fold score 1/sqrt(t)=1/4
    # into the q side (weights AND bias)
    wq_eff = (Wq / (49.0 * 4.0)).astype(np.float32)
    bq_eff = (bq / 4.0).astype(np.float32)
    wk_eff = (Wk / 49.0).astype(np.float32)
    bk_eff = bk.astype(np.float32)
    # t-major partition order: row i = (t=i//8, c=i%8); attention pairs
    # (i, j) belong to the same channel iff i%8 == j%8.  The mask reaches
    # scoresT through 8 augmentation rows: the K side (lhsT) carries the
    # channel indicator, the Q side (rhs) carries the per-channel -30 mask.
    idx = np.arange(P)
    ind = (np.arange(CG)[:, None] == (idx % CG)[None, :]).astype(np.float32)
    consts = np.zeros((P, 452), dtype=np.float32)
    consts[0:DIN, 128:160] = wq_eff
    consts[0:DIN, 160:192] = wk_eff
    consts[DIN, 128:160] = bq_eff
    consts[DIN, 160:192] = bk_eff
    consts[DOUT : DOUT + CG, 194:322] = MASK_NEG * (1.0 - ind)  # Q-side aug
    consts[DOUT : DOUT + CG, 322:450] = ind                      # K-side aug
    consts[:, 450] = 1.0                                         # ones column
    return consts


def kernel(x, Wq, bq, Wk, bk):
    from concourse.bass_utils import run_bass_kernel_spmd

    x = np.ascontiguousarray(x, dtype=np.float32)
    consts = _host_consts(Wq, bq, Wk, bk)

    nc = _build_nc()
    in_maps = [{"x": x[i], "consts": consts} for i in range(N_CORES)]
    res = run_bass_kernel_spmd(nc, in_maps, core_ids=list(range(N_CORES)))
    global LAST_RUN
    LAST_RUN = res
    out = np.stack([r["out"] for r in res.results], axis=0)
    return out


LAST_RUN = None
